# revision 4
# baseline (speedup 1.0000x reference)
"""AttentiveQuantizer forward kernel for Trainium2 (8 NeuronCores, Bass/Tile).

Computes, for latent [16,256,32,32], codebook [4096,256], wq/wk/wv [256,256]:
    q = x @ wq.T            (x = latent NHWC)
    k = codebook @ wk.T
    v = codebook @ wv.T
    logit = q @ k.T / sqrt(C)
    idx = argmax(logit, -1); code = uint8(idx)
    quantized = v[idx] in NCHW     (straight-through estimator == hard_v)

The softmax/soft_v of the reference cancels in the forward value
(stop_gradient(hard - soft) + soft == hard), so it is not computed.

Sharding: data-parallel over N (2 images per core); codebook + weights
replicated. No collectives.
"""

from contextlib import ExitStack

import numpy as np

import concourse.bass as bass
import concourse.tile as tile
from concourse import bacc, mybir
from concourse.bass_utils import run_bass_kernel_spmd
from concourse.masks import make_identity

F32 = mybir.dt.float32
U32 = mybir.dt.uint32

N_CORES = 8
N, C, H, W, K = 16, 256, 32, 32, 4096
NP = N // N_CORES          # images per core = 2
HW = H * W                 # 1024
M = NP * HW                # rows per core = 2048
MT = M // 128              # m-tiles per core = 16
JT = K // 512              # j-chunks = 8
P = 128

_NC_CACHE = {}


def _emit(tc: tile.TileContext):
    nc = tc.nc
    xT = nc.dram_tensor("xT", [NP, C, HW], F32, kind="ExternalInput")
    cbT = nc.dram_tensor("cbT", [C, K], F32, kind="ExternalInput")
    wqT = nc.dram_tensor("wqT", [C, C], F32, kind="ExternalInput")
    wkT = nc.dram_tensor("wkT", [C, C], F32, kind="ExternalInput")
    wvT = nc.dram_tensor("wvT", [C, C], F32, kind="ExternalInput")
    logit_o = nc.dram_tensor("logit_o", [M, K], F32, kind="ExternalOutput")
    idx_o = nc.dram_tensor("idx_o", [P, MT], U32, kind="ExternalOutput")
    quant_o = nc.dram_tensor("quant_o", [NP, C, HW], F32, kind="ExternalOutput")
    v_o = nc.dram_tensor("v_o", [K, C], F32, kind="ExternalOutput")

    with ExitStack() as ctx:
        const = ctx.enter_context(tc.tile_pool(name="const", bufs=1))
        vpool = ctx.enter_context(tc.tile_pool(name="vpool", bufs=2))
        lpool = ctx.enter_context(tc.tile_pool(name="lpool", bufs=2))
        spool = ctx.enter_context(tc.tile_pool(name="spool", bufs=4))
        gpool = ctx.enter_context(tc.tile_pool(name="gpool", bufs=2))
        qzpool = ctx.enter_context(tc.tile_pool(name="qzpool", bufs=1))
        psum_mm = ctx.enter_context(tc.tile_pool(name="psum_mm", bufs=4, space="PSUM"))
        psum_sm = ctx.enter_context(tc.tile_pool(name="psum_sm", bufs=2, space="PSUM"))
        psum_tr = ctx.enter_context(tc.tile_pool(name="psum_tr", bufs=2, space="PSUM"))

        ident = const.tile([P, P], F32)
        make_identity(nc, ident[:])

        # ---- stage 0: load inputs to SBUF ----
        # weight layout [p, ck, c'] with contraction index c = ck*128 + p
        wq_sb = const.tile([P, 2, C], F32, tag="wq_sb")
        wk_sb = const.tile([P, 2, C], F32, tag="wk_sb")
        wv_sb = const.tile([P, 2, C], F32, tag="wv_sb")
        nc.sync.dma_start(wq_sb[:], wqT.ap().rearrange("(k p) c -> p k c", p=P))
        nc.sync.dma_start(wk_sb[:], wkT.ap().rearrange("(k p) c -> p k c", p=P))
        nc.sync.dma_start(wv_sb[:], wvT.ap().rearrange("(k p) c -> p k c", p=P))
        cb_sb = const.tile([P, 2, K], F32, tag="cb_sb")  # [p, ck, j]
        nc.sync.dma_start(cb_sb[:], cbT.ap().rearrange("(k p) j -> p k j", p=P))
        x_sb = const.tile([P, 2, NP, HW], F32, tag="x_sb")  # [p, ck, n, m]
        for n in range(NP):
            nc.sync.dma_start(
                x_sb[:, :, n, :],
                xT.ap()[n].rearrange("(k p) m -> p k m", p=P))

        # ---- stage 1: kT[c', j] = wk @ cb.T ----
        kT_sb = const.tile([P, 2, K], F32, tag="kT_sb")  # [p(c'), ckp, j]
        for ckp in range(2):
            for jt in range(JT):
                ps = psum_sm.tile([P, 512], F32, tag="ps_small")
                for ck in range(2):
                    nc.tensor.matmul(
                        ps[:],
                        wk_sb[:, ck, ckp * P:(ckp + 1) * P],
                        cb_sb[:, ck, jt * 512:(jt + 1) * 512],
                        start=(ck == 0), stop=(ck == 1),
                    )
                nc.scalar.copy(kT_sb[:, ckp, jt * 512:(jt + 1) * 512], ps[:])

        # ---- stage 2: v[j, c'] = cb @ wv.T, streamed to DRAM ----
        for g in range(8):  # groups of 512 codebook rows
            vt = vpool.tile([P, 4, C], F32, tag="vt")
            for s in range(4):
                jt128 = g * 4 + s
                ps = psum_sm.tile([P, 512], F32, tag="ps_small")
                for ck in range(2):
                    nc.tensor.matmul(
                        ps[:, :C],
                        cb_sb[:, ck, jt128 * P:(jt128 + 1) * P],
                        wv_sb[:, ck, :],
                        start=(ck == 0), stop=(ck == 1),
                    )
                nc.scalar.copy(vt[:, s, :], ps[:, :C])
            nc.sync.dma_start(
                v_o.ap()[g * 512:(g + 1) * 512, :].rearrange("(t p) c -> p t c", p=P),
                vt[:],
            )

        # ---- stage 3: qT[c', m] = (wq/16) @ x.T ----
        qT_sb = const.tile([P, 2, M], F32, tag="qT_sb")  # [p(c'), ckp, m]
        for n in range(NP):
            for ckp in range(2):
                for mt in range(2):
                    ps = psum_sm.tile([P, 512], F32, tag="ps_small")
                    for ck in range(2):
                        nc.tensor.matmul(
                            ps[:],
                            wq_sb[:, ck, ckp * P:(ckp + 1) * P],
                            x_sb[:, ck, n, mt * 512:(mt + 1) * 512],
                            start=(ck == 0), stop=(ck == 1),
                        )
                    nc.scalar.copy(
                        qT_sb[:, ckp, n * HW + mt * 512: n * HW + (mt + 1) * 512],
                        ps[:],
                    )

        # ---- stage 4: logit tiles + argmax ----
        idx_sb = const.tile([P, MT], U32, tag="idx_sb")
        for t in range(MT):
            lt = lpool.tile([P, K], F32, tag="lt")
            for jt in range(JT):
                ps = psum_mm.tile([P, 512], F32, tag="ps_mm")
                for ckp in range(2):
                    nc.tensor.matmul(
                        ps[:],
                        qT_sb[:, ckp, t * P:(t + 1) * P],
                        kT_sb[:, ckp, jt * 512:(jt + 1) * 512],
                        start=(ckp == 0), stop=(ckp == 1),
                    )
                nc.scalar.copy(lt[:, jt * 512:(jt + 1) * 512], ps[:])
            nc.sync.dma_start(logit_o.ap()[t * P:(t + 1) * P, :], lt[:])
            mx = spool.tile([P, 8], F32, tag="mx")
            nc.vector.max(out=mx[:], in_=lt[:])
            ix = spool.tile([P, 8], U32, tag="ix")
            nc.vector.max_index(ix[:], mx[:], lt[:])
            nc.vector.tensor_copy(idx_sb[:, t:t + 1], ix[:, 0:1])
        nc.sync.dma_start(idx_o.ap(), idx_sb[:])

        # ---- stage 5: gather v[idx] and transpose to NCHW ----
        qz = {}
        for n in range(NP):
            for ckc in range(2):
                qz[(n, ckc)] = qzpool.tile([P, HW], F32, tag=f"qz_{n}_{ckc}", name=f"qz_{n}_{ckc}")
        for t in range(MT):
            n, col = t // 8, (t % 8) * P
            gt = gpool.tile([P, C], F32, tag="gt")
            nc.gpsimd.indirect_dma_start(
                out=gt[:], out_offset=None, in_=v_o.ap(),
                in_offset=bass.IndirectOffsetOnAxis(ap=idx_sb[:, t:t + 1], axis=0),
            )
            for ckc in range(2):
                pt = psum_tr.tile([P, P], F32, tag="pt")
                nc.tensor.transpose(
                    out=pt[:], in_=gt[:, ckc * P:(ckc + 1) * P], identity=ident[:])
                nc.scalar.copy(qz[(n, ckc)][:, col:col + P], pt[:])
        for n in range(NP):
            for ckc in range(2):
                nc.sync.dma_start(
                    quant_o.ap()[n, ckc * P:(ckc + 1) * P, :], qz[(n, ckc)][:])


def _get_nc():
    if "nc" not in _NC_CACHE:
        nc = bacc.Bacc("TRN2", target_bir_lowering=False, debug=False,
                       num_devices=N_CORES)
        with tile.TileContext(nc) as tc:
            _emit(tc)
        nc.compile()
        _NC_CACHE["nc"] = nc
    return _NC_CACHE["nc"]


def _make_in_maps(latent, codebook, wq, wk, wv):
    latent = np.ascontiguousarray(latent, dtype=np.float32)
    # fold the 1/sqrt(C) logit scale into wq (exact: power of two)
    wqT = np.ascontiguousarray(wq.T.astype(np.float32) / np.float32(16.0))
    wkT = np.ascontiguousarray(wk.T.astype(np.float32))
    wvT = np.ascontiguousarray(wv.T.astype(np.float32))
    cbT = np.ascontiguousarray(codebook.T.astype(np.float32))
    in_maps = []
    for c in range(N_CORES):
        xT = np.ascontiguousarray(
            latent[c * NP:(c + 1) * NP].reshape(NP, C, HW))
        in_maps.append({"xT": xT, "cbT": cbT, "wqT": wqT, "wkT": wkT,
                        "wvT": wvT})
    return in_maps


def _assemble(results):
    logit = np.concatenate(
        [r["logit_o"].reshape(NP, H, W, K) for r in results], axis=0)
    idx = np.concatenate(
        [r["idx_o"].T.reshape(NP, H, W) for r in results], axis=0)
    code = idx.astype(np.uint8)
    quant = np.concatenate(
        [r["quant_o"].reshape(NP, C, H, W) for r in results], axis=0)
    v = results[0]["v_o"]
    return quant, code, logit, v


def kernel(latent, codebook, wq, wk, wv, temperature=1):
    nc = _get_nc()
    in_maps = _make_in_maps(latent, codebook, wq, wk, wv)
    res = run_bass_kernel_spmd(nc, in_maps, core_ids=list(range(N_CORES)))
    return _assemble(res.results)


# revision 8
# speedup vs baseline: 7.2394x; 7.2394x over previous
"""AttentiveQuantizer forward kernel for Trainium2 (8 NeuronCores, Bass/Tile).

Computes, for latent [16,256,32,32], codebook [4096,256], wq/wk/wv [256,256]:
    q = x @ wq.T            (x = latent NHWC)
    k = codebook @ wk.T
    v = codebook @ wv.T
    logit = q @ k.T / sqrt(C)
    idx = argmax(logit, -1); code = uint8(idx)
    quantized = v[idx] in NCHW     (straight-through estimator == hard_v)

The softmax/soft_v of the reference cancels in the forward value
(stop_gradient(hard - soft) + soft == hard), so it is not computed.

Sharding: data-parallel over N (2 images per core); codebook + weights
replicated. No collectives.

Two modes:
  mode="refine" (default): the big logit matmul runs in float32r (full PE
    rate). The argmax is then re-derived exactly: top-8 candidates per row
    from the f32r logits, the top CAND candidates' exact fp32 logits are
    recomputed (gather k rows + dot on gpsimd) and the best is selected.
    logit output carries ~1.5e-4 relative noise; argmax/code/quantized are
    fp32-exact.
  mode="f32": everything in fp32 (slower, logit accurate to ~1e-7).
"""

from contextlib import ExitStack

import numpy as np

import concourse.bass as bass
import concourse.tile as tile
from concourse import bacc, mybir
from concourse.bass_utils import run_bass_kernel_spmd
from concourse.masks import make_identity

F32 = mybir.dt.float32
F32R = mybir.dt.float32r
U32 = mybir.dt.uint32

N_CORES = 8
N, C, H, W, K = 16, 256, 32, 32, 4096
NP = N // N_CORES          # images per core = 2
HW = H * W                 # 1024
M = NP * HW                # rows per core = 2048
MT = M // 128              # m-tiles per core = 16
JT = K // 512              # j-chunks = 8
P = 128
CAND = 4                   # refined argmax candidates per row

_NC_CACHE = {}

DEFAULT_STAGES = frozenset({"logit", "ldma", "scan", "gather"})


def _emit(tc: tile.TileContext, reps: int = 1, stages: frozenset = DEFAULT_STAGES,
          mode: str = "refine"):
    nc = tc.nc
    refine = mode == "refine"
    xT = nc.dram_tensor("xT", [NP, C, HW], F32, kind="ExternalInput")
    cbT = nc.dram_tensor("cbT", [C, K], F32, kind="ExternalInput")
    wqT = nc.dram_tensor("wqT", [C, C], F32, kind="ExternalInput")
    wkT = nc.dram_tensor("wkT", [C, C], F32, kind="ExternalInput")
    wvT = nc.dram_tensor("wvT", [C, C], F32, kind="ExternalInput")
    logit_o = nc.dram_tensor("logit_o", [M, K], F32, kind="ExternalOutput")
    idx_o = nc.dram_tensor("idx_o", [P, MT], U32, kind="ExternalOutput")
    quant_o = nc.dram_tensor("quant_o", [NP, C, HW], F32, kind="ExternalOutput")
    v_o = nc.dram_tensor("v_o", [K, C], F32, kind="ExternalOutput")
    if refine:
        krows_d = nc.dram_tensor("krows_d", [K, C], F32, kind="Internal")

    with ExitStack() as ctx:
        const = ctx.enter_context(tc.tile_pool(name="const", bufs=1))
        vpool = ctx.enter_context(tc.tile_pool(name="vpool", bufs=2))
        lpool = ctx.enter_context(tc.tile_pool(name="lpool", bufs=2))
        spool = ctx.enter_context(tc.tile_pool(name="spool", bufs=4))
        gpool = ctx.enter_context(tc.tile_pool(name="gpool", bufs=2))
        qzpool = ctx.enter_context(tc.tile_pool(name="qzpool", bufs=1))
        psum_mm = ctx.enter_context(tc.tile_pool(name="psum_mm", bufs=3, space="PSUM"))
        psum_sm = ctx.enter_context(tc.tile_pool(name="psum_sm", bufs=3, space="PSUM"))
        psum_tr = ctx.enter_context(tc.tile_pool(name="psum_tr", bufs=2, space="PSUM"))

        ident = const.tile([P, P], F32)
        make_identity(nc, ident[:])

        # ---- stage 0: load inputs to SBUF ----
        # weight layout [p, ck, c'] with contraction index c = ck*128 + p
        wq_sb = const.tile([P, 2, C], F32, tag="wq_sb")
        wk_sb = const.tile([P, 2, C], F32, tag="wk_sb")
        wv_sb = const.tile([P, 2, C], F32, tag="wv_sb")
        nc.sync.dma_start(wq_sb[:], wqT.ap().rearrange("(k p) c -> p k c", p=P))
        nc.sync.dma_start(wk_sb[:], wkT.ap().rearrange("(k p) c -> p k c", p=P))
        nc.sync.dma_start(wv_sb[:], wvT.ap().rearrange("(k p) c -> p k c", p=P))

        for _rep in range(reps):
            # cb_sb shares its slot with the later kT_r (same tag).
            cb_sb = const.tile([P, 2, K], F32, tag="big_a", name="cb_sb")
            nc.sync.dma_start(cb_sb[:], cbT.ap().rearrange("(k p) j -> p k j", p=P))
            x_sb = const.tile([P, 2, NP * HW], F32, tag="big_b", name="x_sb")
            for n in range(NP):
                nc.sync.dma_start(
                    x_sb[:, :, n * HW:(n + 1) * HW],
                    xT.ap()[n].rearrange("(k p) m -> p k m", p=P))

            # ---- stage 1: kT[c', j] = wk @ cb.T (fp32 exact) ----
            kT_sb = const.tile([P, 2, K], F32, tag="kT_sb")  # [p(c'), ckp, j]
            for ckp in range(2):
                for jt in range(JT):
                    ps = psum_sm.tile([P, 512], F32, tag="ps_small")
                    for ck in range(2):
                        nc.tensor.matmul(
                            ps[:],
                            wk_sb[:, ck, ckp * P:(ckp + 1) * P],
                            cb_sb[:, ck, jt * 512:(jt + 1) * 512],
                            start=(ck == 0), stop=(ck == 1),
                        )
                    nc.scalar.copy(kT_sb[:, ckp, jt * 512:(jt + 1) * 512], ps[:])

            # ---- stage 2: v[j, c'] = cb @ wv.T (fp32), streamed to DRAM ----
            for g in range(8):  # groups of 512 codebook rows
                vt = vpool.tile([P, 4, C], F32, tag="vt")
                for s in range(4):
                    jt128 = g * 4 + s
                    ps = psum_sm.tile([P, 512], F32, tag="ps_small")
                    for ck in range(2):
                        nc.tensor.matmul(
                            ps[:, :C],
                            cb_sb[:, ck, jt128 * P:(jt128 + 1) * P],
                            wv_sb[:, ck, :],
                            start=(ck == 0), stop=(ck == 1),
                        )
                    nc.scalar.copy(vt[:, s, :], ps[:, :C])
                nc.sync.dma_start(
                    v_o.ap()[g * 512:(g + 1) * 512, :].rearrange(
                        "(t p) c -> p t c", p=P),
                    vt[:],
                )

            # ---- stage 3: qT[c', m] = (wq/16) @ x.T (fp32 exact) ----
            qT_sb = const.tile([P, 2, M], F32, tag="qT_sb")  # [p(c'), ckp, m]
            for n in range(NP):
                for ckp in range(2):
                    for mt in range(2):
                        ps = psum_sm.tile([P, 512], F32, tag="ps_small")
                        for ck in range(2):
                            nc.tensor.matmul(
                                ps[:],
                                wq_sb[:, ck, ckp * P:(ckp + 1) * P],
                                x_sb[:, ck,
                                     n * HW + mt * 512:n * HW + (mt + 1) * 512],
                                start=(ck == 0), stop=(ck == 1),
                            )
                        nc.scalar.copy(
                            qT_sb[:, ckp,
                                  n * HW + mt * 512: n * HW + (mt + 1) * 512],
                            ps[:],
                        )

            if refine:
                # ---- stage 3b: k rows to DRAM via PE transpose of kT ----
                for g in range(8):
                    krt = vpool.tile([P, 4, C], F32, tag="vt", name="krt")
                    for s in range(4):
                        j128 = g * 4 + s
                        for ckp in range(2):
                            pt = psum_tr.tile([P, P], F32, tag="pt")
                            nc.tensor.transpose(
                                out=pt[:],
                                in_=kT_sb[:, ckp, j128 * P:(j128 + 1) * P],
                                identity=ident[:])
                            nc.scalar.copy(krt[:, s, ckp * P:(ckp + 1) * P],
                                           pt[:])
                    nc.sync.dma_start(
                        krows_d.ap()[g * 512:(g + 1) * 512, :].rearrange(
                            "(t p) c -> p t c", p=P),
                        krt[:],
                    )

                # ---- stage 3c: rounded f32r copies for the fast matmul ----
                # (reuse the cb_sb / x_sb slots, now dead)
                kT_r = const.tile([P, 2, K], F32R, tag="big_a", name="kT_r")
                nc.scalar.copy(kT_r[:], kT_sb[:])
                qT_r = const.tile([P, 2, M], F32R, tag="big_b", name="qT_r")
                nc.scalar.copy(qT_r[:], qT_sb[:])
                qT_mm, kT_mm = qT_r, kT_r
            else:
                qT_mm, kT_mm = qT_sb, kT_sb

            # ---- stage 4: logit tiles + argmax (+ refine) ----
            idx_sb = const.tile([P, MT], U32, tag="idx_sb")
            if refine:
                cvb = const.tile([P, MT, CAND], F32, tag="cvb")
                ixb = const.tile([P, MT, 8], U32, tag="ixb")
            for t in range(MT):
                lt = lpool.tile([P, K], F32, tag="lt")
                if "scan" not in stages:
                    nc.vector.memset(idx_sb[:, t:t + 1], 0)
                for jt in range(JT if "logit" in stages else 0):
                    ps = psum_mm.tile([P, 512], F32, tag="ps_mm")
                    for ckp in range(2):
                        nc.tensor.matmul(
                            ps[:],
                            qT_mm[:, ckp, t * P:(t + 1) * P],
                            kT_mm[:, ckp, jt * 512:(jt + 1) * 512],
                            start=(ckp == 0), stop=(ckp == 1),
                        )
                    nc.scalar.copy(lt[:, jt * 512:(jt + 1) * 512], ps[:])
                if "ldma" in stages:
                    nc.sync.dma_start(logit_o.ap()[t * P:(t + 1) * P, :], lt[:])
                if "scan" not in stages:
                    continue
                mx = spool.tile([P, 8], F32, tag="mx")
                nc.vector.max(out=mx[:], in_=lt[:])
                if not refine:
                    ix = spool.tile([P, 8], U32, tag="ix")
                    nc.vector.max_index(ix[:], mx[:], lt[:])
                    nc.vector.tensor_copy(idx_sb[:, t:t + 1], ix[:, 0:1])
                    continue
                ix = ixb[:, t, :]
                nc.vector.max_index(ix, mx[:], lt[:])

                # exact fp32 logits for the top CAND candidates
                qr = gpool.tile([P, C], F32, tag="qr")
                for ckp in range(2):
                    pt = psum_tr.tile([P, P], F32, tag="pt")
                    nc.tensor.transpose(
                        out=pt[:], in_=qT_sb[:, ckp, t * P:(t + 1) * P],
                        identity=ident[:])
                    nc.scalar.copy(qr[:, ckp * P:(ckp + 1) * P], pt[:])
                kg = gpool.tile([P, CAND, C], F32, tag="kg")
                for c in range(CAND):
                    nc.gpsimd.indirect_dma_start(
                        out=kg[:, c, :], out_offset=None, in_=krows_d.ap(),
                        in_offset=bass.IndirectOffsetOnAxis(
                            ap=ixb[:, t, c:c + 1], axis=0),
                    )
                prod = gpool.tile([P, CAND, C], F32, tag="prod")
                for c in range(CAND):
                    nc.gpsimd.tensor_tensor(
                        out=prod[:, c, :], in0=kg[:, c, :], in1=qr[:],
                        op=mybir.AluOpType.mult)
                nc.vector.tensor_reduce(
                    cvb[:, t, :], prod[:], axis=mybir.AxisListType.X,
                    op=mybir.AluOpType.add)
            if refine and "scan" in stages:
                # batched selection: best exact candidate per row, ties ->
                # smallest index (matches jnp.argmax)
                bvb = spool.tile([P, MT], F32, tag="bvb")
                nc.vector.tensor_reduce(
                    bvb[:], cvb[:], axis=mybir.AxisListType.X,
                    op=mybir.AluOpType.max)
                maskn = spool.tile([P, MT, CAND], F32, tag="maskn")
                nc.vector.tensor_tensor(
                    out=maskn[:], in0=cvb[:],
                    in1=bvb[:].to_broadcast([P, MT, CAND]),
                    op=mybir.AluOpType.not_equal)
                ixf = spool.tile([P, MT, CAND], F32, tag="ixf")
                nc.vector.tensor_copy(ixf[:], ixb[:, :, :CAND])
                sel = spool.tile([P, MT, CAND], F32, tag="sel")
                nc.vector.scalar_tensor_tensor(
                    out=sel[:], in0=maskn[:], scalar=65536.0, in1=ixf[:],
                    op0=mybir.AluOpType.mult, op1=mybir.AluOpType.add)
                bestb = spool.tile([P, MT], F32, tag="bestb")
                nc.vector.tensor_reduce(
                    bestb[:], sel[:], axis=mybir.AxisListType.X,
                    op=mybir.AluOpType.min)
                nc.vector.tensor_copy(idx_sb[:], bestb[:])
            nc.sync.dma_start(idx_o.ap(), idx_sb[:])

            # ---- stage 5: gather v[idx] and transpose to NCHW ----
            qz = {}
            for n in range(NP):
                for ckc in range(2):
                    qz[(n, ckc)] = qzpool.tile(
                        [P, HW], F32, tag=f"qz_{n}_{ckc}", name=f"qz_{n}_{ckc}")
            for t in range(MT if "gather" in stages else 0):
                n, col = t // 8, (t % 8) * P
                gt = gpool.tile([P, C], F32, tag="gt")
                nc.gpsimd.indirect_dma_start(
                    out=gt[:], out_offset=None, in_=v_o.ap(),
                    in_offset=bass.IndirectOffsetOnAxis(
                        ap=idx_sb[:, t:t + 1], axis=0),
                )
                for ckc in range(2):
                    pt = psum_tr.tile([P, P], F32, tag="pt")
                    nc.tensor.transpose(
                        out=pt[:], in_=gt[:, ckc * P:(ckc + 1) * P],
                        identity=ident[:])
                    nc.scalar.copy(qz[(n, ckc)][:, col:col + P], pt[:])
            for n in range(NP if "gather" in stages else 0):
                for ckc in range(2):
                    nc.sync.dma_start(
                        quant_o.ap()[n, ckc * P:(ckc + 1) * P, :],
                        qz[(n, ckc)][:])


KH = K // 2  # codebook half width for SBUF streaming


def _emit_v3(tc: tile.TileContext, reps: int = 1,
             stages: frozenset = DEFAULT_STAGES):
    """All-f32r pipeline. Host supplies hi/lo 12-bit mantissa splits of every
    operand; exact fp32 tables (k rows, q rows, v) are built from 3-term
    compensated f32r matmuls (hi*hi + hi*lo + lo*hi; the dropped lo*lo term
    is ~2^-24 relative). The big logit matmul is single-term (hi*hi, then
    f32r-rounded tables) -> ~2e-4 logit noise; argmax is refined exactly."""
    nc = tc.nc
    xh_d = nc.dram_tensor("xh", [NP, C, HW], F32R, kind="ExternalInput")
    xl_d = nc.dram_tensor("xl", [NP, C, HW], F32R, kind="ExternalInput")
    cbh_d = nc.dram_tensor("cbh", [C, K], F32R, kind="ExternalInput")
    cbl_d = nc.dram_tensor("cbl", [C, K], F32R, kind="ExternalInput")
    wname = {}
    for w in ("wq", "wk", "wv"):
        for h in ("h", "l"):
            wname[w + h] = nc.dram_tensor(w + h, [C, C], F32R,
                                          kind="ExternalInput")
    logit_o = nc.dram_tensor("logit_o", [M, K], F32, kind="ExternalOutput")
    idx_o = nc.dram_tensor("idx_o", [P, MT], U32, kind="ExternalOutput")
    quant_o = nc.dram_tensor("quant_o", [NP, C, HW], F32, kind="ExternalOutput")
    v_o = nc.dram_tensor("v_o", [K, C], F32, kind="ExternalOutput")
    krows_d = nc.dram_tensor("krows_d", [K, C], F32, kind="Internal")

    with ExitStack() as ctx:
        const = ctx.enter_context(tc.tile_pool(name="const", bufs=1))
        vpool = ctx.enter_context(tc.tile_pool(name="vpool", bufs=2))
        lpool = ctx.enter_context(tc.tile_pool(name="lpool", bufs=2))
        spool = ctx.enter_context(tc.tile_pool(name="spool", bufs=4))
        gpool = ctx.enter_context(tc.tile_pool(name="gpool", bufs=2))
        qdpool = ctx.enter_context(tc.tile_pool(name="qdpool", bufs=3))
        psum_mm = ctx.enter_context(tc.tile_pool(name="psum_mm", bufs=3,
                                                 space="PSUM"))
        psum_sm = ctx.enter_context(tc.tile_pool(name="psum_sm", bufs=3,
                                                 space="PSUM"))
        psum_tr = ctx.enter_context(tc.tile_pool(name="psum_tr", bufs=2,
                                                 space="PSUM"))

        ident = const.tile([P, P], F32)
        make_identity(nc, ident[:])

        wsb = {}
        for nm, dram in wname.items():
            wsb[nm] = const.tile([P, 2, C], F32R, tag=f"w_{nm}", name=f"w_{nm}")
            nc.sync.dma_start(wsb[nm][:],
                              dram.ap().rearrange("(k p) c -> p k c", p=P))

        def load_cb_half(src, jh, tag, name):
            t = const.tile([P, 2, KH], F32R, tag=tag, name=name)
            nc.sync.dma_start(
                t[:], src.ap()[:, jh * KH:(jh + 1) * KH].rearrange(
                    "(k p) j -> p k j", p=P))
            return t

        for _rep in range(reps):
            # ---- prefix: kT_r (hi-only) and qT_r (hi-only) ----
            kT_r = const.tile([P, 2, K], F32R, tag="kT_r")
            for jh in range(2):
                ch = load_cb_half(cbh_d, jh, "cbhi", f"cbhi{jh}_{_rep}")
                for ckp in range(2):
                    for jt in range(4):
                        off = jt * 512
                        ps = psum_sm.tile([P, 512], F32, tag="ps_small")
                        for ck in range(2):
                            nc.tensor.matmul(
                                ps[:],
                                wsb["wkh"][:, ck, ckp * P:(ckp + 1) * P],
                                ch[:, ck, off:off + 512],
                                start=(ck == 0), stop=(ck == 1))
                        nc.scalar.copy(
                            kT_r[:, ckp, jh * KH + off:jh * KH + off + 512],
                            ps[:])
            x_hi = const.tile([P, 2, M], F32R, tag="x_hi")
            for n in range(NP):
                nc.sync.dma_start(
                    x_hi[:, :, n * HW:(n + 1) * HW],
                    xh_d.ap()[n].rearrange("(k p) m -> p k m", p=P))
            qT_r = const.tile([P, 2, M], F32R, tag="qT_r")
            for n in range(NP):
                for ckp in range(2):
                    for mt in range(2):
                        ps = psum_sm.tile([P, 512], F32, tag="ps_small")
                        for ck in range(2):
                            nc.tensor.matmul(
                                ps[:],
                                wsb["wqh"][:, ck, ckp * P:(ckp + 1) * P],
                                x_hi[:, ck,
                                     n * HW + mt * 512:n * HW + (mt + 1) * 512],
                                start=(ck == 0), stop=(ck == 1))
                        nc.scalar.copy(
                            qT_r[:, ckp,
                                 n * HW + mt * 512:n * HW + (mt + 1) * 512],
                            ps[:])

            # ---- stage 4a: logit tiles + scans ----
            idx_sb = const.tile([P, MT], U32, tag="idx_sb")
            cvb = const.tile([P, MT, CAND], F32, tag="cvb")
            ixb = const.tile([P, MT, 8], U32, tag="ixb")
            for t in range(MT):
                lt = lpool.tile([P, K], F32, tag="lt")
                if "scan" not in stages:
                    nc.vector.memset(idx_sb[:, t:t + 1], 0)
                for jt in range(JT if "logit" in stages else 0):
                    ps = psum_mm.tile([P, 512], F32, tag="ps_mm")
                    for ckp in range(2):
                        nc.tensor.matmul(
                            ps[:],
                            qT_r[:, ckp, t * P:(t + 1) * P],
                            kT_r[:, ckp, jt * 512:(jt + 1) * 512],
                            start=(ckp == 0), stop=(ckp == 1))
                    nc.scalar.copy(lt[:, jt * 512:(jt + 1) * 512], ps[:])
                if "ldma" in stages:
                    nc.sync.dma_start(logit_o.ap()[t * P:(t + 1) * P, :], lt[:])
                if "scan" in stages:
                    mx = spool.tile([P, 8], F32, tag="mx")
                    nc.vector.max(out=mx[:], in_=lt[:])
                    nc.vector.max_index(ixb[:, t, :], mx[:], lt[:])

            # ---- exact tables: k rows, then q rows, then v ----
            def table_rows(dst_dram, whi, wlo, jh, ch, cl):
                # dst rows [j, c'] for half jh via 3-term compensation
                for g in range(4):  # groups of 4 x 128 rows
                    krt = vpool.tile([P, 4, C], F32, tag="vt", name="tt")
                    for s in range(4):
                        off = (g * 4 + s) * P
                        ps = psum_sm.tile([P, 512], F32, tag="ps_small")
                        first = True
                        for ck in range(2):
                            for (a, b) in ((ch, whi), (ch, wlo), (cl, whi)):
                                nc.tensor.matmul(
                                    ps[:, :C],
                                    a[:, ck, off:off + P],
                                    b[:, ck, :],
                                    start=first,
                                    stop=(ck == 1 and b is whi and a is cl))
                                first = False
                        nc.scalar.copy(krt[:, s, :], ps[:, :C])
                    nc.sync.dma_start(
                        dst_dram.ap()[jh * KH + g * 512:
                                      jh * KH + (g + 1) * 512, :].rearrange(
                            "(t p) c -> p t c", p=P),
                        krt[:])

            for jh in range(2):
                ch = load_cb_half(cbh_d, jh, "cbhi", f"cbhi2_{jh}_{_rep}")
                cl = load_cb_half(cbl_d, jh, "cblo", f"cblo_{jh}_{_rep}")
                table_rows(krows_d, wsb["wkh"], wsb["wkl"], jh, ch, cl)

            x_lo = const.tile([P, 2, M], F32R, tag="cblo", name=f"x_lo_{_rep}")
            for n in range(NP):
                nc.sync.dma_start(
                    x_lo[:, :, n * HW:(n + 1) * HW],
                    xl_d.ap()[n].rearrange("(k p) m -> p k m", p=P))
            qrows = const.tile([P, MT, C], F32, tag="qrows")
            for t in range(MT):
                ps = psum_sm.tile([P, 512], F32, tag="ps_small")
                first = True
                for ck in range(2):
                    for (a, b) in ((x_hi, wsb["wqh"]), (x_hi, wsb["wql"]),
                                   (x_lo, wsb["wqh"])):
                        nc.tensor.matmul(
                            ps[:, :C],
                            a[:, ck, t * P:(t + 1) * P],
                            b[:, ck, :],
                            start=first,
                            stop=(ck == 1 and b is wsb["wqh"] and a is x_lo))
                        first = False
                nc.scalar.copy(qrows[:, t, :], ps[:, :C])

            for jh in range(2):
                ch = load_cb_half(cbh_d, jh, "cbhi", f"cbhi3_{jh}_{_rep}")
                cl = load_cb_half(cbl_d, jh, "cblo", f"cblo2_{jh}_{_rep}")
                table_rows(v_o, wsb["wvh"], wsb["wvl"], jh, ch, cl)

            # ---- stage 4b: refine dots + batched selection ----
            if "scan" in stages:
                for t in range(MT):
                    kg = gpool.tile([P, CAND, C], F32, tag="kg")
                    for c in range(CAND):
                        nc.gpsimd.indirect_dma_start(
                            out=kg[:, c, :], out_offset=None,
                            in_=krows_d.ap(),
                            in_offset=bass.IndirectOffsetOnAxis(
                                ap=ixb[:, t, c:c + 1], axis=0))
                    prod = gpool.tile([P, CAND, C], F32, tag="prod")
                    for c in range(CAND):
                        nc.gpsimd.tensor_tensor(
                            out=prod[:, c, :], in0=kg[:, c, :],
                            in1=qrows[:, t, :], op=mybir.AluOpType.mult)
                    nc.vector.tensor_reduce(
                        cvb[:, t, :], prod[:], axis=mybir.AxisListType.X,
                        op=mybir.AluOpType.add)
                bvb = spool.tile([P, MT], F32, tag="bvb")
                nc.vector.tensor_reduce(
                    bvb[:], cvb[:], axis=mybir.AxisListType.X,
                    op=mybir.AluOpType.max)
                maskn = spool.tile([P, MT, CAND], F32, tag="maskn")
                nc.vector.tensor_tensor(
                    out=maskn[:], in0=cvb[:],
                    in1=bvb[:].to_broadcast([P, MT, CAND]),
                    op=mybir.AluOpType.not_equal)
                ixf = spool.tile([P, MT, CAND], F32, tag="ixf")
                nc.vector.tensor_copy(ixf[:], ixb[:, :, :CAND])
                sel = spool.tile([P, MT, CAND], F32, tag="sel")
                nc.vector.scalar_tensor_tensor(
                    out=sel[:], in0=maskn[:], scalar=65536.0, in1=ixf[:],
                    op0=mybir.AluOpType.mult, op1=mybir.AluOpType.add)
                bestb = spool.tile([P, MT], F32, tag="bestb")
                nc.vector.tensor_reduce(
                    bestb[:], sel[:], axis=mybir.AxisListType.X,
                    op=mybir.AluOpType.min)
                nc.vector.tensor_copy(idx_sb[:], bestb[:])
            nc.sync.dma_start(idx_o.ap(), idx_sb[:])

            # ---- stage 5: gather v[idx], transpose, direct DMA out ----
            for t in range(MT if "gather" in stages else 0):
                n, col = t // 8, (t % 8) * P
                gt = gpool.tile([P, C], F32, tag="gt")
                nc.gpsimd.indirect_dma_start(
                    out=gt[:], out_offset=None, in_=v_o.ap(),
                    in_offset=bass.IndirectOffsetOnAxis(
                        ap=idx_sb[:, t:t + 1], axis=0))
                for ckc in range(2):
                    pt = psum_tr.tile([P, P], F32, tag="pt")
                    nc.tensor.transpose(
                        out=pt[:], in_=gt[:, ckc * P:(ckc + 1) * P],
                        identity=ident[:])
                    qd = qdpool.tile([P, P], F32, tag="qd")
                    nc.scalar.copy(qd[:], pt[:])
                    nc.sync.dma_start(
                        quant_o.ap()[n, ckc * P:(ckc + 1) * P, col:col + P],
                        qd[:])


def _get_nc(reps: int = 1, stages=DEFAULT_STAGES, mode: str = "refine"):
    key = ("nc", reps, tuple(sorted(stages)), mode)
    if key not in _NC_CACHE:
        nc = bacc.Bacc("TRN2", target_bir_lowering=False, debug=False,
                       num_devices=N_CORES)
        with tile.TileContext(nc) as tc:
            if mode == "v3":
                _emit_v3(tc, reps=reps, stages=frozenset(stages))
            else:
                _emit(tc, reps=reps, stages=frozenset(stages), mode=mode)
        nc.compile()
        _NC_CACHE[key] = nc
    return _NC_CACHE[key]


def _split12(a):
    """Split fp32 array into hi (12-bit mantissa, f32r-exact) + lo = a - hi
    (also f32r-exact). hi uses round-to-nearest at 12 dropped bits."""
    a = np.ascontiguousarray(a, dtype=np.float32)
    b = a.view(np.uint32).astype(np.uint64)
    hi = ((b + (1 << 11)) & ~np.uint64((1 << 12) - 1)).astype(np.uint32)
    hi = hi.view(np.float32).reshape(a.shape)
    lo = (a - hi).astype(np.float32)
    return hi, lo


def _make_in_maps_v3(latent, codebook, wq, wk, wv):
    latent = np.ascontiguousarray(latent, dtype=np.float32)
    wqs = wq.T.astype(np.float32) / np.float32(16.0)
    wqh, wql = _split12(np.ascontiguousarray(wqs))
    wkh, wkl = _split12(np.ascontiguousarray(wk.T.astype(np.float32)))
    wvh, wvl = _split12(np.ascontiguousarray(wv.T.astype(np.float32)))
    cbh, cbl = _split12(np.ascontiguousarray(codebook.T.astype(np.float32)))
    in_maps = []
    for c in range(N_CORES):
        xT = np.ascontiguousarray(
            latent[c * NP:(c + 1) * NP].reshape(NP, C, HW))
        xh, xl = _split12(xT)
        in_maps.append({"xh": xh, "xl": xl, "cbh": cbh, "cbl": cbl,
                        "wqh": wqh, "wql": wql, "wkh": wkh, "wkl": wkl,
                        "wvh": wvh, "wvl": wvl})
    return in_maps


def _make_in_maps(latent, codebook, wq, wk, wv):
    latent = np.ascontiguousarray(latent, dtype=np.float32)
    # fold the 1/sqrt(C) logit scale into wq (exact: power of two)
    wqT = np.ascontiguousarray(wq.T.astype(np.float32) / np.float32(16.0))
    wkT = np.ascontiguousarray(wk.T.astype(np.float32))
    wvT = np.ascontiguousarray(wv.T.astype(np.float32))
    cbT = np.ascontiguousarray(codebook.T.astype(np.float32))
    in_maps = []
    for c in range(N_CORES):
        xT = np.ascontiguousarray(
            latent[c * NP:(c + 1) * NP].reshape(NP, C, HW))
        in_maps.append({"xT": xT, "cbT": cbT, "wqT": wqT, "wkT": wkT,
                        "wvT": wvT})
    return in_maps


def _assemble(results):
    logit = np.concatenate(
        [r["logit_o"].reshape(NP, H, W, K) for r in results], axis=0)
    idx = np.concatenate(
        [r["idx_o"].T.reshape(NP, H, W) for r in results], axis=0)
    code = idx.astype(np.uint8)
    quant = np.concatenate(
        [r["quant_o"].reshape(NP, C, H, W) for r in results], axis=0)
    v = results[0]["v_o"]
    return quant, code, logit, v


MODE = "v3"


def kernel(latent, codebook, wq, wk, wv, temperature=1):
    nc = _get_nc(mode=MODE)
    if MODE == "v3":
        in_maps = _make_in_maps_v3(latent, codebook, wq, wk, wv)
    else:
        in_maps = _make_in_maps(latent, codebook, wq, wk, wv)
    res = run_bass_kernel_spmd(nc, in_maps, core_ids=list(range(N_CORES)))
    return _assemble(res.results)


# revision 13
# speedup vs baseline: 9.2305x; 1.2750x over previous
"""AttentiveQuantizer forward kernel for Trainium2 (8 NeuronCores, Bass/Tile).

Computes, for latent [16,256,32,32], codebook [4096,256], wq/wk/wv [256,256]:
    q = x @ wq.T            (x = latent NHWC)
    k = codebook @ wk.T
    v = codebook @ wv.T
    logit = q @ k.T / sqrt(C)
    idx = argmax(logit, -1); code = uint8(idx)
    quantized = v[idx] in NCHW     (straight-through estimator == hard_v)

The softmax/soft_v of the reference cancels in the forward value
(stop_gradient(hard - soft) + soft == hard), so it is not computed.

Sharding: data-parallel over N (2 images per core); codebook + weights
replicated. No collectives.

Two modes:
  mode="refine" (default): the big logit matmul runs in float32r (full PE
    rate). The argmax is then re-derived exactly: top-8 candidates per row
    from the f32r logits, the top CAND candidates' exact fp32 logits are
    recomputed (gather k rows + dot on gpsimd) and the best is selected.
    logit output carries ~1.5e-4 relative noise; argmax/code/quantized are
    fp32-exact.
  mode="f32": everything in fp32 (slower, logit accurate to ~1e-7).
"""

from contextlib import ExitStack

import numpy as np

import concourse.bass as bass
import concourse.tile as tile
from concourse import bacc, mybir
from concourse.bass_utils import run_bass_kernel_spmd
from concourse.masks import make_identity

F32 = mybir.dt.float32
F32R = mybir.dt.float32r
U32 = mybir.dt.uint32

N_CORES = 8
N, C, H, W, K = 16, 256, 32, 32, 4096
NP = N // N_CORES          # images per core = 2
HW = H * W                 # 1024
M = NP * HW                # rows per core = 2048
MT = M // 128              # m-tiles per core = 16
JT = K // 512              # j-chunks = 8
P = 128
CAND = 4                   # refined argmax candidates per row

_NC_CACHE = {}

DEFAULT_STAGES = frozenset({"logit", "ldma", "scan", "gather"})


def _emit(tc: tile.TileContext, reps: int = 1, stages: frozenset = DEFAULT_STAGES,
          mode: str = "refine"):
    nc = tc.nc
    refine = mode == "refine"
    xT = nc.dram_tensor("xT", [NP, C, HW], F32, kind="ExternalInput")
    cbT = nc.dram_tensor("cbT", [C, K], F32, kind="ExternalInput")
    wqT = nc.dram_tensor("wqT", [C, C], F32, kind="ExternalInput")
    wkT = nc.dram_tensor("wkT", [C, C], F32, kind="ExternalInput")
    wvT = nc.dram_tensor("wvT", [C, C], F32, kind="ExternalInput")
    logit_o = nc.dram_tensor("logit_o", [M, K], F32, kind="ExternalOutput")
    idx_o = nc.dram_tensor("idx_o", [P, MT], U32, kind="ExternalOutput")
    quant_o = nc.dram_tensor("quant_o", [NP, C, HW], F32, kind="ExternalOutput")
    v_o = nc.dram_tensor("v_o", [K, C], F32, kind="ExternalOutput")
    if refine:
        krows_d = nc.dram_tensor("krows_d", [K, C], F32, kind="Internal")

    with ExitStack() as ctx:
        const = ctx.enter_context(tc.tile_pool(name="const", bufs=1))
        vpool = ctx.enter_context(tc.tile_pool(name="vpool", bufs=2))
        lpool = ctx.enter_context(tc.tile_pool(name="lpool", bufs=2))
        spool = ctx.enter_context(tc.tile_pool(name="spool", bufs=4))
        gpool = ctx.enter_context(tc.tile_pool(name="gpool", bufs=2))
        qzpool = ctx.enter_context(tc.tile_pool(name="qzpool", bufs=1))
        psum_mm = ctx.enter_context(tc.tile_pool(name="psum_mm", bufs=3, space="PSUM"))
        psum_sm = ctx.enter_context(tc.tile_pool(name="psum_sm", bufs=3, space="PSUM"))
        psum_tr = ctx.enter_context(tc.tile_pool(name="psum_tr", bufs=2, space="PSUM"))

        ident = const.tile([P, P], F32)
        make_identity(nc, ident[:])

        # ---- stage 0: load inputs to SBUF ----
        # weight layout [p, ck, c'] with contraction index c = ck*128 + p
        wq_sb = const.tile([P, 2, C], F32, tag="wq_sb")
        wk_sb = const.tile([P, 2, C], F32, tag="wk_sb")
        wv_sb = const.tile([P, 2, C], F32, tag="wv_sb")
        nc.sync.dma_start(wq_sb[:], wqT.ap().rearrange("(k p) c -> p k c", p=P))
        nc.sync.dma_start(wk_sb[:], wkT.ap().rearrange("(k p) c -> p k c", p=P))
        nc.sync.dma_start(wv_sb[:], wvT.ap().rearrange("(k p) c -> p k c", p=P))

        for _rep in range(reps):
            # cb_sb shares its slot with the later kT_r (same tag).
            cb_sb = const.tile([P, 2, K], F32, tag="big_a", name="cb_sb")
            nc.sync.dma_start(cb_sb[:], cbT.ap().rearrange("(k p) j -> p k j", p=P))
            x_sb = const.tile([P, 2, NP * HW], F32, tag="big_b", name="x_sb")
            for n in range(NP):
                nc.sync.dma_start(
                    x_sb[:, :, n * HW:(n + 1) * HW],
                    xT.ap()[n].rearrange("(k p) m -> p k m", p=P))

            # ---- stage 1: kT[c', j] = wk @ cb.T (fp32 exact) ----
            kT_sb = const.tile([P, 2, K], F32, tag="kT_sb")  # [p(c'), ckp, j]
            for ckp in range(2):
                for jt in range(JT):
                    ps = psum_sm.tile([P, 512], F32, tag="ps_small")
                    for ck in range(2):
                        nc.tensor.matmul(
                            ps[:],
                            wk_sb[:, ck, ckp * P:(ckp + 1) * P],
                            cb_sb[:, ck, jt * 512:(jt + 1) * 512],
                            start=(ck == 0), stop=(ck == 1),
                        )
                    nc.scalar.copy(kT_sb[:, ckp, jt * 512:(jt + 1) * 512], ps[:])

            # ---- stage 2: v[j, c'] = cb @ wv.T (fp32), streamed to DRAM ----
            for g in range(8):  # groups of 512 codebook rows
                vt = vpool.tile([P, 4, C], F32, tag="vt")
                for s in range(4):
                    jt128 = g * 4 + s
                    ps = psum_sm.tile([P, 512], F32, tag="ps_small")
                    for ck in range(2):
                        nc.tensor.matmul(
                            ps[:, :C],
                            cb_sb[:, ck, jt128 * P:(jt128 + 1) * P],
                            wv_sb[:, ck, :],
                            start=(ck == 0), stop=(ck == 1),
                        )
                    nc.scalar.copy(vt[:, s, :], ps[:, :C])
                nc.sync.dma_start(
                    v_o.ap()[g * 512:(g + 1) * 512, :].rearrange(
                        "(t p) c -> p t c", p=P),
                    vt[:],
                )

            # ---- stage 3: qT[c', m] = (wq/16) @ x.T (fp32 exact) ----
            qT_sb = const.tile([P, 2, M], F32, tag="qT_sb")  # [p(c'), ckp, m]
            for n in range(NP):
                for ckp in range(2):
                    for mt in range(2):
                        ps = psum_sm.tile([P, 512], F32, tag="ps_small")
                        for ck in range(2):
                            nc.tensor.matmul(
                                ps[:],
                                wq_sb[:, ck, ckp * P:(ckp + 1) * P],
                                x_sb[:, ck,
                                     n * HW + mt * 512:n * HW + (mt + 1) * 512],
                                start=(ck == 0), stop=(ck == 1),
                            )
                        nc.scalar.copy(
                            qT_sb[:, ckp,
                                  n * HW + mt * 512: n * HW + (mt + 1) * 512],
                            ps[:],
                        )

            if refine:
                # ---- stage 3b: k rows to DRAM via PE transpose of kT ----
                for g in range(8):
                    krt = vpool.tile([P, 4, C], F32, tag="vt", name="krt")
                    for s in range(4):
                        j128 = g * 4 + s
                        for ckp in range(2):
                            pt = psum_tr.tile([P, P], F32, tag="pt")
                            nc.tensor.transpose(
                                out=pt[:],
                                in_=kT_sb[:, ckp, j128 * P:(j128 + 1) * P],
                                identity=ident[:])
                            nc.scalar.copy(krt[:, s, ckp * P:(ckp + 1) * P],
                                           pt[:])
                    nc.sync.dma_start(
                        krows_d.ap()[g * 512:(g + 1) * 512, :].rearrange(
                            "(t p) c -> p t c", p=P),
                        krt[:],
                    )

                # ---- stage 3c: rounded f32r copies for the fast matmul ----
                # (reuse the cb_sb / x_sb slots, now dead)
                kT_r = const.tile([P, 2, K], F32R, tag="big_a", name="kT_r")
                nc.scalar.copy(kT_r[:], kT_sb[:])
                qT_r = const.tile([P, 2, M], F32R, tag="big_b", name="qT_r")
                nc.scalar.copy(qT_r[:], qT_sb[:])
                qT_mm, kT_mm = qT_r, kT_r
            else:
                qT_mm, kT_mm = qT_sb, kT_sb

            # ---- stage 4: logit tiles + argmax (+ refine) ----
            idx_sb = const.tile([P, MT], U32, tag="idx_sb")
            if refine:
                cvb = const.tile([P, MT, CAND], F32, tag="cvb")
                ixb = const.tile([P, MT, 8], U32, tag="ixb")
            for t in range(MT):
                lt = lpool.tile([P, K], F32, tag="lt")
                if "scan" not in stages:
                    nc.vector.memset(idx_sb[:, t:t + 1], 0)
                for jt in range(JT if "logit" in stages else 0):
                    ps = psum_mm.tile([P, 512], F32, tag="ps_mm")
                    for ckp in range(2):
                        nc.tensor.matmul(
                            ps[:],
                            qT_mm[:, ckp, t * P:(t + 1) * P],
                            kT_mm[:, ckp, jt * 512:(jt + 1) * 512],
                            start=(ckp == 0), stop=(ckp == 1),
                        )
                    nc.scalar.copy(lt[:, jt * 512:(jt + 1) * 512], ps[:])
                if "ldma" in stages:
                    nc.sync.dma_start(logit_o.ap()[t * P:(t + 1) * P, :], lt[:])
                if "scan" not in stages:
                    continue
                mx = spool.tile([P, 8], F32, tag="mx")
                nc.vector.max(out=mx[:], in_=lt[:])
                if not refine:
                    ix = spool.tile([P, 8], U32, tag="ix")
                    nc.vector.max_index(ix[:], mx[:], lt[:])
                    nc.vector.tensor_copy(idx_sb[:, t:t + 1], ix[:, 0:1])
                    continue
                ix = ixb[:, t, :]
                nc.vector.max_index(ix, mx[:], lt[:])

                # exact fp32 logits for the top CAND candidates
                qr = gpool.tile([P, C], F32, tag="qr")
                for ckp in range(2):
                    pt = psum_tr.tile([P, P], F32, tag="pt")
                    nc.tensor.transpose(
                        out=pt[:], in_=qT_sb[:, ckp, t * P:(t + 1) * P],
                        identity=ident[:])
                    nc.scalar.copy(qr[:, ckp * P:(ckp + 1) * P], pt[:])
                kg = gpool.tile([P, CAND, C], F32, tag="kg")
                for c in range(CAND):
                    nc.gpsimd.indirect_dma_start(
                        out=kg[:, c, :], out_offset=None, in_=krows_d.ap(),
                        in_offset=bass.IndirectOffsetOnAxis(
                            ap=ixb[:, t, c:c + 1], axis=0),
                    )
                prod = gpool.tile([P, CAND, C], F32, tag="prod")
                for c in range(CAND):
                    nc.gpsimd.tensor_tensor(
                        out=prod[:, c, :], in0=kg[:, c, :], in1=qr[:],
                        op=mybir.AluOpType.mult)
                nc.vector.tensor_reduce(
                    cvb[:, t, :], prod[:], axis=mybir.AxisListType.X,
                    op=mybir.AluOpType.add)
            if refine and "scan" in stages:
                # batched selection: best exact candidate per row, ties ->
                # smallest index (matches jnp.argmax)
                bvb = spool.tile([P, MT], F32, tag="bvb")
                nc.vector.tensor_reduce(
                    bvb[:], cvb[:], axis=mybir.AxisListType.X,
                    op=mybir.AluOpType.max)
                maskn = spool.tile([P, MT, CAND], F32, tag="maskn")
                nc.vector.tensor_tensor(
                    out=maskn[:], in0=cvb[:],
                    in1=bvb[:].to_broadcast([P, MT, CAND]),
                    op=mybir.AluOpType.not_equal)
                ixf = spool.tile([P, MT, CAND], F32, tag="ixf")
                nc.vector.tensor_copy(ixf[:], ixb[:, :, :CAND])
                sel = spool.tile([P, MT, CAND], F32, tag="sel")
                nc.vector.scalar_tensor_tensor(
                    out=sel[:], in0=maskn[:], scalar=65536.0, in1=ixf[:],
                    op0=mybir.AluOpType.mult, op1=mybir.AluOpType.add)
                bestb = spool.tile([P, MT], F32, tag="bestb")
                nc.vector.tensor_reduce(
                    bestb[:], sel[:], axis=mybir.AxisListType.X,
                    op=mybir.AluOpType.min)
                nc.vector.tensor_copy(idx_sb[:], bestb[:])
            nc.sync.dma_start(idx_o.ap(), idx_sb[:])

            # ---- stage 5: gather v[idx] and transpose to NCHW ----
            qz = {}
            for n in range(NP):
                for ckc in range(2):
                    qz[(n, ckc)] = qzpool.tile(
                        [P, HW], F32, tag=f"qz_{n}_{ckc}", name=f"qz_{n}_{ckc}")
            for t in range(MT if "gather" in stages else 0):
                n, col = t // 8, (t % 8) * P
                gt = gpool.tile([P, C], F32, tag="gt")
                nc.gpsimd.indirect_dma_start(
                    out=gt[:], out_offset=None, in_=v_o.ap(),
                    in_offset=bass.IndirectOffsetOnAxis(
                        ap=idx_sb[:, t:t + 1], axis=0),
                )
                for ckc in range(2):
                    pt = psum_tr.tile([P, P], F32, tag="pt")
                    nc.tensor.transpose(
                        out=pt[:], in_=gt[:, ckc * P:(ckc + 1) * P],
                        identity=ident[:])
                    nc.scalar.copy(qz[(n, ckc)][:, col:col + P], pt[:])
            for n in range(NP if "gather" in stages else 0):
                for ckc in range(2):
                    nc.sync.dma_start(
                        quant_o.ap()[n, ckc * P:(ckc + 1) * P, :],
                        qz[(n, ckc)][:])


KH = K // 2  # codebook half width for SBUF streaming


def _emit_v3(tc: tile.TileContext, reps: int = 1,
             stages: frozenset = DEFAULT_STAGES):
    """All-f32r pipeline. Host supplies hi/lo 12-bit mantissa splits of every
    operand; exact fp32 tables (kT, qT, vT) are built from 3-term compensated
    f32r matmuls at N=512 (hi*hi + hi*lo + lo*hi; dropped lo*lo ~2^-24 rel),
    row-layout tables derived via PE transposes. The big logit matmul is
    single-rounded f32r (~2e-4 noise); argmax is refined exactly from the
    fp32 tables."""
    nc = tc.nc
    xh_d = nc.dram_tensor("xh", [NP, C, HW], F32R, kind="ExternalInput")
    xl_d = nc.dram_tensor("xl", [NP, C, HW], F32R, kind="ExternalInput")
    cbh_d = nc.dram_tensor("cbh", [C, K], F32R, kind="ExternalInput")
    cbl_d = nc.dram_tensor("cbl", [C, K], F32R, kind="ExternalInput")
    wname = {}
    for w in ("wq", "wk", "wv"):
        for h in ("h", "l"):
            wname[w + h] = nc.dram_tensor(w + h, [C, C], F32R,
                                          kind="ExternalInput")
    logit_o = nc.dram_tensor("logit_o", [M, K], F32, kind="ExternalOutput")
    idx_o = nc.dram_tensor("idx_o", [P, MT], U32, kind="ExternalOutput")
    quant_o = nc.dram_tensor("quant_o", [NP, C, HW], F32, kind="ExternalOutput")
    v_o = nc.dram_tensor("v_o", [K, C], F32, kind="ExternalOutput")
    krows_d = nc.dram_tensor("krows_d", [K, C], F32, kind="Internal")

    with ExitStack() as ctx:
        const = ctx.enter_context(tc.tile_pool(name="const", bufs=1))
        vpool = ctx.enter_context(tc.tile_pool(name="vpool", bufs=1))
        lpool = ctx.enter_context(tc.tile_pool(name="lpool", bufs=2))
        spool = ctx.enter_context(tc.tile_pool(name="spool", bufs=2))
        gpool = ctx.enter_context(tc.tile_pool(name="gpool", bufs=2))
        kgpool = ctx.enter_context(tc.tile_pool(name="kgpool", bufs=1))
        prodpool = ctx.enter_context(tc.tile_pool(name="prodpool", bufs=1))
        qdpool = ctx.enter_context(tc.tile_pool(name="qdpool", bufs=2))
        # (gt shares the kg slot region via its own small pool)
        psum_mm = ctx.enter_context(tc.tile_pool(name="psum_mm", bufs=3,
                                                 space="PSUM"))
        psum_sm = ctx.enter_context(tc.tile_pool(name="psum_sm", bufs=3,
                                                 space="PSUM"))
        psum_tr = ctx.enter_context(tc.tile_pool(name="psum_tr", bufs=2,
                                                 space="PSUM"))

        ident = const.tile([P, P], F32)
        make_identity(nc, ident[:])

        wsb = {}
        for nm, dram in wname.items():
            wsb[nm] = const.tile([P, 2, C], F32R, tag=f"w_{nm}", name=f"w_{nm}")
            nc.sync.dma_start(wsb[nm][:],
                              dram.ap().rearrange("(k p) c -> p k c", p=P))

        def load_half(src, jh, tag, name):
            t = const.tile([P, 2, KH], F32R, tag=tag, name=name)
            nc.sync.dma_start(
                t[:], src.ap()[:, jh * KH:(jh + 1) * KH].rearrange(
                    "(k p) j -> p k j", p=P))
            return t

        def comp_cols(dst, whi, wlo, hi_t, lo_t, width, dst_off):
            """compensated col-layout: dst[c', dst_off + 0:width] for one
            512-chunk at a time. hi_t/lo_t free width = `width`."""
            for ckp in range(2):
                for jt in range(width // 512):
                    off = jt * 512
                    ps = psum_sm.tile([P, 512], F32, tag="ps_small")
                    first = True
                    for ck in range(2):
                        for (a, b) in ((hi_t, whi), (hi_t, wlo), (lo_t, whi)):
                            nc.tensor.matmul(
                                ps[:],
                                b[:, ck, ckp * P:(ckp + 1) * P],
                                a[:, ck, off:off + 512],
                                start=first,
                                stop=(ck == 1 and a is lo_t))
                            first = False
                    nc.scalar.copy(
                        dst[:, ckp, dst_off + off:dst_off + off + 512], ps[:])

        def rows_from_cols(colT, dst_dram, nrow):
            """transpose col-layout [c', j] (fp32) into row chunks and DMA
            to dst_dram [nrow, C]."""
            for g in range(nrow // 512):
                krt = vpool.tile([P, 4, C], F32, tag="vt", name="rt")
                for s in range(4):
                    j128 = g * 4 + s
                    for ckp in range(2):
                        pt = psum_tr.tile([P, P], F32, tag="pt")
                        nc.tensor.transpose(
                            out=pt[:],
                            in_=colT[:, ckp, j128 * P:(j128 + 1) * P],
                            identity=ident[:])
                        nc.scalar.copy(krt[:, s, ckp * P:(ckp + 1) * P], pt[:])
                nc.sync.dma_start(
                    dst_dram.ap()[g * 512:(g + 1) * 512, :].rearrange(
                        "(t p) c -> p t c", p=P),
                    krt[:])

        for _rep in range(reps):
            # ---- kT exact (col layout) + rounded kT_r + k rows ----
            kT_ex = const.tile([P, 2, K], F32, tag="big_a", name="kT_ex")
            for jh in range(2):
                ch = load_half(cbh_d, jh, "half_h", f"cbh{jh}_{_rep}")
                cl = load_half(cbl_d, jh, "half_l", f"cbl{jh}_{_rep}")
                comp_cols(kT_ex, wsb["wkh"], wsb["wkl"], ch, cl, KH, jh * KH)
            kT_r = const.tile([P, 2, K], F32R, tag="kT_r")
            nc.scalar.copy(kT_r[:], kT_ex[:])
            rows_from_cols(kT_ex, krows_d, K)

            # ---- qT exact + rounded qT_r + q rows (kept in SBUF) ----
            xh_t = const.tile([P, 2, M], F32R, tag="half_h", name=f"xh_{_rep}")
            xl_t = const.tile([P, 2, M], F32R, tag="half_l", name=f"xl_{_rep}")
            for n in range(NP):
                nc.sync.dma_start(
                    xh_t[:, :, n * HW:(n + 1) * HW],
                    xh_d.ap()[n].rearrange("(k p) m -> p k m", p=P))
                nc.sync.dma_start(
                    xl_t[:, :, n * HW:(n + 1) * HW],
                    xl_d.ap()[n].rearrange("(k p) m -> p k m", p=P))
            qT_ex = const.tile([P, 2, M], F32, tag="qT_ex")
            comp_cols(qT_ex, wsb["wqh"], wsb["wql"], xh_t, xl_t, M, 0)
            qT_r = const.tile([P, 2, M], F32R, tag="qT_r")
            nc.scalar.copy(qT_r[:], qT_ex[:])
            qrows = const.tile([P, MT, C], F32, tag="qrows")
            for t in range(MT):
                for ckp in range(2):
                    pt = psum_tr.tile([P, P], F32, tag="pt")
                    nc.tensor.transpose(
                        out=pt[:], in_=qT_ex[:, ckp, t * P:(t + 1) * P],
                        identity=ident[:])
                    nc.scalar.copy(qrows[:, t, ckp * P:(ckp + 1) * P], pt[:])

            # ---- stage 4a: logit tiles + scans ----
            idx_sb = const.tile([P, MT], U32, tag="idx_sb")
            cvb = const.tile([P, MT, CAND], F32, tag="cvb")
            ixb = const.tile([P, MT, 8], U32, tag="ixb")
            for t in range(MT):
                lt = lpool.tile([P, K], F32, tag="lt")
                if "scan" not in stages:
                    nc.vector.memset(idx_sb[:, t:t + 1], 0)
                for jt in range(JT if "logit" in stages else 0):
                    ps = psum_mm.tile([P, 512], F32, tag="ps_mm")
                    for ckp in range(2):
                        nc.tensor.matmul(
                            ps[:],
                            qT_r[:, ckp, t * P:(t + 1) * P],
                            kT_r[:, ckp, jt * 512:(jt + 1) * 512],
                            start=(ckp == 0), stop=(ckp == 1))
                    nc.scalar.copy(lt[:, jt * 512:(jt + 1) * 512], ps[:])
                if "ldma" in stages:
                    nc.sync.dma_start(logit_o.ap()[t * P:(t + 1) * P, :], lt[:])
                if "scan" in stages:
                    mx = spool.tile([P, 8], F32, tag="mx")
                    nc.vector.max(out=mx[:], in_=lt[:])
                    nc.vector.max_index(ixb[:, t, :], mx[:], lt[:])

            # ---- stage 4b: refine dots (overlap with vT below) ----
            if "scan" in stages:
                for t in range(MT):
                    kg = kgpool.tile([P, CAND, C], F32, tag="kg")
                    for c in range(CAND):
                        nc.gpsimd.indirect_dma_start(
                            out=kg[:, c, :], out_offset=None,
                            in_=krows_d.ap(),
                            in_offset=bass.IndirectOffsetOnAxis(
                                ap=ixb[:, t, c:c + 1], axis=0))
                    prod = prodpool.tile([P, CAND, C], F32, tag="prod")
                    for c in range(CAND):
                        nc.gpsimd.tensor_tensor(
                            out=prod[:, c, :], in0=kg[:, c, :],
                            in1=qrows[:, t, :], op=mybir.AluOpType.mult)
                    nc.vector.tensor_reduce(
                        cvb[:, t, :], prod[:], axis=mybir.AxisListType.X,
                        op=mybir.AluOpType.add)

            # ---- vT exact + v rows to DRAM (PE work after logit) ----
            for jh in range(2):
                ch = load_half(cbh_d, jh, "half_h", f"cbh2{jh}_{_rep}")
                cl = load_half(cbl_d, jh, "half_l", f"cbl2{jh}_{_rep}")
                vT_ex = const.tile([P, 2, KH], F32, tag="qT_ex",
                                   name=f"vT_{jh}_{_rep}")
                comp_cols(vT_ex, wsb["wvh"], wsb["wvl"], ch, cl, KH, 0)
                for g in range(4):
                    krt = vpool.tile([P, 4, C], F32, tag="vt", name="vrt")
                    for s in range(4):
                        j128 = g * 4 + s
                        for ckp in range(2):
                            pt = psum_tr.tile([P, P], F32, tag="pt")
                            nc.tensor.transpose(
                                out=pt[:],
                                in_=vT_ex[:, ckp, j128 * P:(j128 + 1) * P],
                                identity=ident[:])
                            nc.scalar.copy(krt[:, s, ckp * P:(ckp + 1) * P],
                                           pt[:])
                    nc.sync.dma_start(
                        v_o.ap()[jh * KH + g * 512:
                                 jh * KH + (g + 1) * 512, :].rearrange(
                            "(t p) c -> p t c", p=P),
                        krt[:])

            # ---- batched selection ----
            if "scan" in stages:
                bvb = spool.tile([P, MT], F32, tag="bvb")
                nc.vector.tensor_reduce(
                    bvb[:], cvb[:], axis=mybir.AxisListType.X,
                    op=mybir.AluOpType.max)
                maskn = spool.tile([P, MT, CAND], F32, tag="maskn")
                nc.vector.tensor_tensor(
                    out=maskn[:], in0=cvb[:],
                    in1=bvb[:].to_broadcast([P, MT, CAND]),
                    op=mybir.AluOpType.not_equal)
                ixf = spool.tile([P, MT, CAND], F32, tag="ixf")
                nc.vector.tensor_copy(ixf[:], ixb[:, :, :CAND])
                sel = spool.tile([P, MT, CAND], F32, tag="sel")
                nc.vector.scalar_tensor_tensor(
                    out=sel[:], in0=maskn[:], scalar=65536.0, in1=ixf[:],
                    op0=mybir.AluOpType.mult, op1=mybir.AluOpType.add)
                bestb = spool.tile([P, MT], F32, tag="bestb")
                nc.vector.tensor_reduce(
                    bestb[:], sel[:], axis=mybir.AxisListType.X,
                    op=mybir.AluOpType.min)
                nc.vector.tensor_copy(idx_sb[:], bestb[:])
            nc.sync.dma_start(idx_o.ap(), idx_sb[:])

            # ---- stage 5: gather v[idx], transpose, direct DMA out ----
            for t in range(MT if "gather" in stages else 0):
                n, col = t // 8, (t % 8) * P
                gt = gpool.tile([P, C], F32, tag="gt")
                nc.gpsimd.indirect_dma_start(
                    out=gt[:], out_offset=None, in_=v_o.ap(),
                    in_offset=bass.IndirectOffsetOnAxis(
                        ap=idx_sb[:, t:t + 1], axis=0))
                for ckc in range(2):
                    pt = psum_tr.tile([P, P], F32, tag="pt")
                    nc.tensor.transpose(
                        out=pt[:], in_=gt[:, ckc * P:(ckc + 1) * P],
                        identity=ident[:])
                    qd = qdpool.tile([P, P], F32, tag="qd")
                    nc.scalar.copy(qd[:], pt[:])
                    nc.sync.dma_start(
                        quant_o.ap()[n, ckc * P:(ckc + 1) * P, col:col + P],
                        qd[:])


def _get_nc(reps: int = 1, stages=DEFAULT_STAGES, mode: str = "refine"):
    key = ("nc", reps, tuple(sorted(stages)), mode)
    if key not in _NC_CACHE:
        nc = bacc.Bacc("TRN2", target_bir_lowering=False, debug=False,
                       num_devices=N_CORES)
        with tile.TileContext(nc) as tc:
            if mode == "v3":
                _emit_v3(tc, reps=reps, stages=frozenset(stages))
            else:
                _emit(tc, reps=reps, stages=frozenset(stages), mode=mode)
        nc.compile()
        _NC_CACHE[key] = nc
    return _NC_CACHE[key]


def _split12(a):
    """Split fp32 array into hi (12-bit mantissa, f32r-exact) + lo = a - hi
    (also f32r-exact). hi uses round-to-nearest at 12 dropped bits."""
    a = np.ascontiguousarray(a, dtype=np.float32)
    b = a.view(np.uint32).astype(np.uint64)
    hi = ((b + (1 << 11)) & ~np.uint64((1 << 12) - 1)).astype(np.uint32)
    hi = hi.view(np.float32).reshape(a.shape)
    lo = (a - hi).astype(np.float32)
    return hi, lo


def _make_in_maps_v3(latent, codebook, wq, wk, wv):
    latent = np.ascontiguousarray(latent, dtype=np.float32)
    wqs = wq.T.astype(np.float32) / np.float32(16.0)
    wqh, wql = _split12(np.ascontiguousarray(wqs))
    wkh, wkl = _split12(np.ascontiguousarray(wk.T.astype(np.float32)))
    wvh, wvl = _split12(np.ascontiguousarray(wv.T.astype(np.float32)))
    cbh, cbl = _split12(np.ascontiguousarray(codebook.T.astype(np.float32)))
    in_maps = []
    for c in range(N_CORES):
        xT = np.ascontiguousarray(
            latent[c * NP:(c + 1) * NP].reshape(NP, C, HW))
        xh, xl = _split12(xT)
        in_maps.append({"xh": xh, "xl": xl, "cbh": cbh, "cbl": cbl,
                        "wqh": wqh, "wql": wql, "wkh": wkh, "wkl": wkl,
                        "wvh": wvh, "wvl": wvl})
    return in_maps


def _make_in_maps(latent, codebook, wq, wk, wv):
    latent = np.ascontiguousarray(latent, dtype=np.float32)
    # fold the 1/sqrt(C) logit scale into wq (exact: power of two)
    wqT = np.ascontiguousarray(wq.T.astype(np.float32) / np.float32(16.0))
    wkT = np.ascontiguousarray(wk.T.astype(np.float32))
    wvT = np.ascontiguousarray(wv.T.astype(np.float32))
    cbT = np.ascontiguousarray(codebook.T.astype(np.float32))
    in_maps = []
    for c in range(N_CORES):
        xT = np.ascontiguousarray(
            latent[c * NP:(c + 1) * NP].reshape(NP, C, HW))
        in_maps.append({"xT": xT, "cbT": cbT, "wqT": wqT, "wkT": wkT,
                        "wvT": wvT})
    return in_maps


def _assemble(results):
    logit = np.concatenate(
        [r["logit_o"].reshape(NP, H, W, K) for r in results], axis=0)
    idx = np.concatenate(
        [r["idx_o"].T.reshape(NP, H, W) for r in results], axis=0)
    code = idx.astype(np.uint8)
    quant = np.concatenate(
        [r["quant_o"].reshape(NP, C, H, W) for r in results], axis=0)
    v = results[0]["v_o"]
    return quant, code, logit, v


MODE = "v3"


def kernel(latent, codebook, wq, wk, wv, temperature=1):
    nc = _get_nc(mode=MODE)
    if MODE == "v3":
        in_maps = _make_in_maps_v3(latent, codebook, wq, wk, wv)
    else:
        in_maps = _make_in_maps(latent, codebook, wq, wk, wv)
    res = run_bass_kernel_spmd(nc, in_maps, core_ids=list(range(N_CORES)))
    return _assemble(res.results)
